# revision 1
# baseline (speedup 1.0000x reference)
"""Trainium2 kernel for nn_DistanceRelativeBias.

Computes out[b,k,i,j] = g_k(||c_i - c_j||) where g_k(d) = b2[k] +
sum_h w2[h,k]*silu(w1[h]*d + b1[h]).

Strategy: the 16 head-functions g_k are scalar functions of the pairwise
distance d. On the host we refit them (near-exactly, residual ~1e-8) onto a
16-term silu basis  g_k(d) = sum_f Q[f,k]*silu(a_f*d + c_f)  chosen by OMP
over a candidate dictionary with a diversity constraint (keeps the TF32
matmul rounding amplification small).

Per core (8 cores: core = (batch b, half h) handles 512 query rows x 1024 keys):
  1. PE  (f32r):  d2[i,j] = -2*c_i.c_j + r_i + r_j via K=13 gram matmul
                  (hi/lo-split operands: f32r speed, fp32 accuracy)
  2. DVE:         clamp d2 >= 0
  3. ACT:         d = sqrt(d2) -> fp16 tile (all sqrts phase-separated from
                  silus: their ACT table sets conflict)
  4. PE  (fp16):  broadcast-matmul: selector scatters 8 query rows across
                  128 partitions (partition p=8f+g holds a_f * d of the
                  group's row g), 8 pairs per moving column; fp16 weights
                  get fast-weight-load
  5. ACT:         phi = silu(. + c_f) with per-partition bias AP -> fp16
  6. PE  (fp16):  heads matmul, block-permuted Q [128x128]:
                  psum[8k+g] = sum_f Q[f,k] phi[8f+g]  (128 outputs/cycle)
  7. DVE:         psum -> sbuf cast to fp16
  8. DMA:         sbuf -> DRAM out[16,512,1024] fp16 (4KB contiguous runs,
                  rotated across SP/ACT HWDGE rings + gpsimd SWDGE);
                  host upcasts to fp32
"""
import numpy as np

B, N, D = 4, 1024, 3
HID, HEADS = 64, 16
NCORES = 8
IPC = N // 2            # i-rows per core (512)
NCHUNK = IPC // 128     # 4 chunks of 128 i-rows
NGRP = 16               # groups of 8 i-rows per chunk
NF = 16                 # basis size (features per pair)

# packed const tensor column layout
_SEL0 = 0                 # 16 selectors [128,128] -> cols [0, 2048)
_QM0 = 2048               # Qmat [128,128]        -> cols [2048, 2176)
_CV0 = 2176               # cvec [128,1]          -> col 2176
_RHS0 = 2177              # rhsD [13,1024]        -> cols [2177, 3201)
_LHS0 = 3201              # lhsD [13,512]         -> cols [3201, 3713)
_CW = 3713
_KD = 13                  # distance matmul contraction (hi/lo split for f32r)


def _round10(x):
    """Round mantissa to 10 bits (conservative f32r grid)."""
    i = np.asarray(x, np.float32).view(np.int32).astype(np.int64)
    r = (i + 0x1000 + ((i >> 13) & 1)) >> 13 << 13
    return (r & 0xFFFFFFFF).astype(np.uint32).view(np.float32)

_prog_cache = {}


def _silu(x):
    return x / (1.0 + np.exp(-x))


def _fit_basis(coords, w1, b1, w2, b2):
    """Fit g_k(d) ~= sum_f Q[f,k] silu(a_f d + c_f). Returns (a[16], c[16], Q[16,16])."""
    c64 = coords.astype(np.float64)
    w1 = w1.astype(np.float64).reshape(-1)
    b1 = b1.astype(np.float64)
    w2 = w2.astype(np.float64)
    b2 = b2.astype(np.float64)

    # distance distribution (exact, f64)
    qs = []
    dmax = 0.0
    for b in range(B):
        cb = c64[b]
        r = (cb * cb).sum(1)
        d2 = np.maximum(r[:, None] + r[None, :] - 2.0 * (cb @ cb.T), 0.0)
        d = np.sqrt(d2)
        dmax = max(dmax, d.max())
        qs.append(np.quantile(d.ravel(), np.linspace(0, 1, 1024)))
    grid = np.sort(np.concatenate([np.linspace(0, dmax * 1.02, 4096)] + qs))

    def g(d):
        return _silu(d[:, None] * w1 + b1) @ w2 + b2

    G = g(grid)

    cand = [(float(w1[h]), float(b1[h])) for h in range(HID)]
    for a in (-2, -1.5, -1.25, -1.0, -0.8, -0.6, -0.45, -0.3,
              0.3, 0.45, 0.6, 0.8, 1.0, 1.25, 1.5, 2.0):
        for c in np.linspace(-9, 9, 37):
            cand.append((a, float(c)))
    cand = np.array(cand)
    # snap slopes to the fp16 grid: the broadcast matmul runs in fp16, so
    # fp16-exact slopes make the selector weights lossless
    cand[:, 0] = np.float16(cand[:, 0]).astype(np.float64)
    Phi = _silu(grid[:, None] * cand[:, 0] + cand[:, 1])
    nrm = np.linalg.norm(Phi, axis=0)

    sel = []
    for _ in range(NF):
        if sel:
            A = Phi[:, sel]
            Qq, _ = np.linalg.qr(A)
            R = G - Qq @ (Qq.T @ G)
            P = Phi - Qq @ (Qq.T @ Phi)
        else:
            R, P = G, Phi
        n = np.linalg.norm(P, axis=0)
        score = np.linalg.norm(P.T @ R, axis=1) / np.maximum(n, 1e-12)
        score[n < 0.02 * nrm] = -1.0   # diversity: skip near-dependent units
        for j in sel:
            score[j] = -1.0
        sel.append(int(np.argmax(score)))

    A = Phi[:, sel]
    s = np.linalg.norm(A, axis=0)
    An = A / s
    Q = np.linalg.solve(An.T @ An + 1e-8 * np.eye(NF), An.T @ G) / s[:, None]
    return cand[sel, 0].copy(), cand[sel, 1].copy(), Q


def _make_cst(coords_b, half, avec, cvec, Q):
    """Per-core packed const array [128, _CW] f32."""
    cst = np.zeros((128, _CW), dtype=np.float32)
    a_perm = np.repeat(avec, 8).astype(np.float32)   # a_perm[p] = a[p//8]
    c_perm = np.repeat(cvec, 8).astype(np.float32)

    # selector (t, s): block t covers chunk rows [16t, 16t+16); partition slot
    # g takes row 16t + 2g + s.  Row-striping by 2 keeps DRAM runs at 4KB for
    # the fp16 output (2 adjacent i-rows x 2KB per partition).
    p = np.arange(128)
    for t in range(8):
        for st in range(2):
            v = 2 * t + st
            S = np.zeros((128, 128), dtype=np.float32)
            S[16 * t + 2 * (p % 8) + st, p] = a_perm[p]
            cst[:, _SEL0 + 128 * v:_SEL0 + 128 * (v + 1)] = S

    # heads matmul weights; output partition m = 8k + g so that the 8 query
    # rows of one head are partition-adjacent -> 32KB contiguous DRAM runs
    Qm = np.zeros((128, 128), dtype=np.float32)
    for f in range(NF):
        for gslot in range(8):
            for k in range(HEADS):
                Qm[8 * f + gslot, 8 * k + gslot] = np.float32(Q[f, k])
    cst[:, _QM0:_QM0 + 128] = Qm
    cst[:, _CV0] = c_perm

    # d2 = -2*c_i.c_j + r_i + r_j with hi/lo split operands so the matmul can
    # run at f32r speed with fp32-level accuracy:
    #   c = ch + cl, r = rh + rl;  -2c_i.c_j ~= -2ch_i.ch_j -2ch_i.cl_j -2cl_i.ch_j
    c64 = coords_b.astype(np.float64)
    cf = c64.astype(np.float32)
    ch = _round10(cf)
    cl = (cf.astype(np.float64) - ch).astype(np.float32)
    r = (c64 * c64).sum(1)
    rf = r.astype(np.float32)
    rh = _round10(rf)
    rl = (rf.astype(np.float64) - rh).astype(np.float32)

    rhsD = np.zeros((_KD, N), dtype=np.float32)
    rhsD[0:3] = ch.T
    rhsD[3:6] = cl.T
    rhsD[6:9] = ch.T
    rhsD[9] = 1.0
    rhsD[10] = 1.0
    rhsD[11] = rh
    rhsD[12] = rl
    cst[0:_KD, _RHS0:_RHS0 + N] = rhsD

    i0 = half * IPC
    sl = slice(i0, i0 + IPC)
    lhsD = np.zeros((_KD, IPC), dtype=np.float32)
    lhsD[0:3] = -2.0 * ch[sl].T
    lhsD[3:6] = -2.0 * ch[sl].T
    lhsD[6:9] = -2.0 * cl[sl].T
    lhsD[9] = rh[sl]
    lhsD[10] = rl[sl]
    lhsD[11] = 1.0
    lhsD[12] = 1.0
    cst[0:_KD, _LHS0:_LHS0 + IPC] = lhsD
    return cst


def _build_program():
    import concourse.bacc as bacc
    import concourse.mybir as mybir
    import concourse.tile as tile
    from concourse.tile_rust import add_dep_helper

    f32 = mybir.dt.float32
    f32r = mybir.dt.float32r
    f16 = mybir.dt.float16
    AF = mybir.ActivationFunctionType

    nc = bacc.Bacc(num_devices=NCORES)
    CST = nc.declare_dram_parameter("cst", [128, _CW], f32, isOutput=False)
    OUT = nc.declare_dram_parameter("out", [HEADS, IPC, N], f16, isOutput=True)

    with tile.TileContext(nc) as tc:
        with (
            tc.tile_pool(name="const", bufs=1) as cp,
            tc.tile_pool(name="dtiles", bufs=NCHUNK) as dp,
            tc.tile_pool(name="wq", bufs=4) as wq,
            tc.tile_pool(name="wphi", bufs=8) as wphi,
            tc.tile_pool(name="wout", bufs=8) as wout,
            tc.tile_pool(name="psA", bufs=2, space="PSUM") as psA,
            tc.tile_pool(name="psB", bufs=2, space="PSUM") as psB,
        ):
            # ---- constants ----
            # small critical region (Qmat+cvec+rhsD+lhsD) first so the d2
            # matmuls start immediately; the 1MB selector block loads behind it
            cst = cp.tile([128, _CW], f32, tag="cst")
            nc.sync.dma_start(cst[0:_KD, _RHS0:_CW], CST[0:_KD, _RHS0:_CW])
            nc.sync.dma_start(cst[:, _QM0:_RHS0], CST[:, _QM0:_RHS0])
            nc.scalar.dma_start(cst[:, _SEL0:_SEL0 + 2048], CST[:, _SEL0:_SEL0 + 2048])
            ddr = cp.tile([_KD, N + IPC], f32r, tag="ddr")
            nc.vector.tensor_copy(ddr[:], cst[0:_KD, _RHS0:_RHS0 + N + IPC])
            qmr = cp.tile([128, 128], f16, tag="qmr")
            nc.vector.tensor_copy(qmr[:], cst[:, _QM0:_QM0 + 128])
            selr = cp.tile([128, 2048], f16, tag="selr")
            nc.vector.tensor_copy(selr[:], cst[:, _SEL0:_SEL0 + 2048])
            cvec = cst[:, _CV0:_CV0 + 1]

            # ---- ACT table warmup (dependency-light) ----
            warm = cp.tile([128, 8], f32, tag="warm")
            nc.gpsimd.memset(warm[:], 0.0)
            warm2 = cp.tile([128, 8], f32, tag="warm2")
            nc.scalar.activation(warm2[:], warm[:], AF.Sqrt)

            # ---- phase 1: distances (all sqrts before any silu: the sqrt and
            # silu ACT table sets cannot coexist, so phase-separate) ----
            d_t = []
            for ic in range(NCHUNK):
                pd = psA.tile([128, N], f32, tag="ps")
                for hh in range(2):
                    nc.tensor.matmul(
                        pd[:, 512 * hh:512 * (hh + 1)],
                        ddr[:, N + 128 * ic:N + 128 * (ic + 1)],
                        ddr[:, 512 * hh:512 * (hh + 1)],
                        start=True, stop=True)
                dsq = wq.tile([128, N], f32, tag="dsq")
                nc.vector.tensor_scalar(dsq[:], pd[:], 0.0, None, mybir.AluOpType.max)
                dt = dp.tile([128, N], f16, tag="dt")
                sq_i = nc.scalar.activation(dt[:], dsq[:], AF.Sqrt)
                d_t.append(dt)
                last_sqrt = sq_i

            # re-arm the silu table; pinned after every sqrt so the scheduler
            # cannot interleave silu/sqrt (their ACT table sets conflict).
            rearm = nc.scalar.activation(warm2[:], d_t[-1][0:128, 0:8], AF.Silu)
            add_dep_helper(rearm.ins, last_sqrt.ins, sync=False,
                           reason="silu table re-arm after all sqrts")

            # ---- phase 2: broadcast -> silu -> heads -> store ----
            # per superblock (32 i-rows): 4 subgroups; heads outputs pair up in
            # one 4-bank psum tile (one wide DVE copy per pair), and the 4
            # subgroups pack one [128, 4096] sbuf tile = one 2MB DMA with 16KB
            # contiguous runs.
            ndma = 0
            for ic in range(NCHUNK):
                dt = d_t[ic]
                for t in range(8):
                    osb = wout.tile([128, 2 * N], f16, tag="osb")
                    for st in range(2):
                        v = 2 * t + st
                        pb = psA.tile([128, N], f32, tag="ps")
                        for hh in range(2):
                            nc.tensor.matmul(
                                pb[:, 512 * hh:512 * (hh + 1)],
                                selr[:, 128 * v:128 * (v + 1)],
                                dt[:, 512 * hh:512 * (hh + 1)],
                                start=True, stop=True)
                        phi = wphi.tile([128, N], f16, tag="phi")
                        si = nc.scalar.activation(phi[:], pb[:], AF.Silu,
                                                  bias=cvec, scale=1.0)
                        add_dep_helper(si.ins, rearm.ins, sync=False,
                                       reason="keep silus after sqrt phase")
                        po = psB.tile([128, N], f32, tag="po")
                        for hh in range(2):
                            nc.tensor.matmul(
                                po[:, 512 * hh:512 * (hh + 1)],
                                qmr[:],
                                phi[:, 512 * hh:512 * (hh + 1)],
                                start=True, stop=True)
                        nc.vector.tensor_copy(osb[:, N * st:N * (st + 1)], po[:])
                    i0 = 128 * ic + 16 * t
                    eng = (nc.sync, nc.scalar, nc.gpsimd, nc.sync, nc.gpsimd)[ndma % 5]
                    ndma += 1
                    eng.dma_start(
                        OUT[:, i0:i0 + 16, :].rearrange(
                            "k (g two) j -> k g (two j)", two=2),
                        osb[:])
    nc.compile()
    return nc


def _run(coords, w1, b1, w2, b2, trace=False):
    from concourse.bass_utils import run_bass_kernel_spmd

    avec, cvec, Q = _fit_basis(coords, w1, b1, w2, b2)
    if "nc" not in _prog_cache:
        _prog_cache["nc"] = _build_program()
    nc = _prog_cache["nc"]

    in_maps = []
    for core in range(NCORES):
        b, h = divmod(core, 2)
        in_maps.append({"cst": _make_cst(np.asarray(coords)[b], h, avec, cvec, Q)})

    res = run_bass_kernel_spmd(nc, in_maps, list(range(NCORES)), trace=trace)

    out = np.empty((B, HEADS, N, N), dtype=np.float32)
    for core in range(NCORES):
        b, h = divmod(core, 2)
        out[b, :, h * IPC:(h + 1) * IPC, :] = res.results[core]["out"].astype(np.float32)
    return out, res


def kernel(coords, w1, b1, w2, b2):
    out, _ = _run(coords, w1, b1, w2, b2, trace=False)
    return out



# revision 4
# speedup vs baseline: 1.5629x; 1.5629x over previous
"""Trainium2 kernel for nn_DistanceRelativeBias.

Computes out[b,k,i,j] = g_k(||c_i - c_j||) where g_k(d) = b2[k] +
sum_h w2[h,k]*silu(w1[h]*d + b1[h]).

Key ideas vs the previous version (115 us):
  1. SYMMETRY: out[b,k,i,j] == out[b,k,j,i], so only the upper-triangle
     128x128 (i,j) tiles are computed on device (36 of 64 per batch);
     the host mirrors the off-diagonal tiles.  144 tiles / 8 cores = 18
     tiles per core, perfectly balanced.  Tile coordinates are baked
     into per-core constant data (the lhs/rhs columns of the d2 matmul),
     so all cores run one SPMD program.
  2. d^2 DOMAIN: the 16 head-functions are refit (runtime VarPro
     Gauss-Newton, residual ~5e-3) onto an NF-term silu basis in
     u = d^2:  g_k ~= sum_f Q[f,k]*silu(a_f*u + c_f).  No sqrt pass,
     no ACT table switch, no clamp (silu of a slightly-negative u is
     benign, unlike sqrt).
  3. NF=4 basis with 32-row groups: silu cost on ACT scales with
     NF (phi elements), so fewer, wider groups cut ACT work ~2.4x.
     (Falls back to NF=8 / 16-row groups if the runtime fit is poor.)

Per core, per 384-column stripe (3 of its 18 tiles):
  PE  : u[i,j] = -2 c_i.c_j + r_i + r_j   (K=13 hi/lo-split f32r matmul)
  DVE : psum -> fp16 u-tile
  per 32-row group v: PE broadcast-matmul (selector scatters rows across
        partitions p=32f+g with weight a_f) -> ACT silu(.+c_f) -> fp16
        phi -> PE heads matmuls (4x 4-head blocks, m=32k+g) ->
        DVE/ACT cast psum -> fp16 osb
  DMA : osb [128, 6144] fp16 -> DRAM (1.5 MB contiguous, 12KB runs)
Host unscrambles (pure data movement) + mirrors + upcasts to fp32.
"""
import numpy as np

B, N, D = 4, 1024, 3
HID, HEADS = 64, 16
NCORES = 8
NT = 18                 # 128x128 tiles per core
NSB = 3                 # tile-subblocks per stripe
NST = NT // NSB         # stripes per core (6)
W = NSB * 128           # stripe width (384)
_KD = 13                # d2 matmul contraction (hi/lo split for f32r)
_AMAX = 24.0            # silu slope bound (fp16-robustness of the basis)

# upper-triangle tile list per batch: 36 tiles; cores 2b, 2b+1 take halves
TILES = [(ci, cj) for ci in range(8) for cj in range(ci, 8)]

_prog_cache = {}


def _round10(x):
    """Round mantissa to 10 bits (conservative f32r grid)."""
    i = np.asarray(x, np.float32).view(np.int32).astype(np.int64)
    r = (i + 0x1000 + ((i >> 13) & 1)) >> 13 << 13
    return (r & 0xFFFFFFFF).astype(np.uint32).view(np.float32)


def _silu(x):
    x = np.clip(x, -60.0, 60.0)
    return x / (1.0 + np.exp(-x))


# ---------------------------------------------------------------- basis fit
def _g_exact(d, w1, b1, w2, b2):
    return _silu(d[..., None] * w1 + b1) @ w2 + b2


def _solveQ(usamp, Gs, gnorm, a, c, lam=1e-10):
    phi = _silu(usamp[:, None] * a + c)
    A = phi.T @ phi + lam * np.eye(a.size)
    try:
        Q = np.linalg.solve(A, phi.T @ Gs)
    except np.linalg.LinAlgError:
        Q = np.linalg.lstsq(phi, Gs, rcond=None)[0]
    return Q, np.linalg.norm(phi @ Q - Gs) / gnorm


def _varpro(usamp, Gs, gnorm, a0, c0, iters=80):
    """Variable-projection Gauss-Newton over (a, c); Q solved exactly."""
    a, c = a0.astype(np.float64).copy(), c0.astype(np.float64).copy()
    NF = a.size
    Q, f = _solveQ(usamp, Gs, gnorm, a, c)
    lm = 1e-3
    for _ in range(iters):
        p = np.concatenate([a, c])
        r0 = (_silu(usamp[:, None] * a + c) @ Q - Gs).ravel()
        J = np.empty((r0.size, 2 * NF))
        for i in range(2 * NF):
            dp = np.zeros(2 * NF)
            dp[i] = max(1e-5, 1e-6 * abs(p[i]))
            a2 = np.clip((p + dp)[:NF], -_AMAX, _AMAX)
            c2 = (p + dp)[NF:]
            Q2, _ = _solveQ(usamp, Gs, gnorm, a2, c2)
            J[:, i] = ((_silu(usamp[:, None] * a2 + c2) @ Q2 - Gs).ravel() - r0) / dp[i]
        JtJ = J.T @ J
        Jtr = J.T @ r0
        ok = False
        for _ in range(10):
            try:
                step = np.linalg.solve(
                    JtJ + lm * np.diag(np.maximum(np.diag(JtJ), 1e-12)), -Jtr)
            except np.linalg.LinAlgError:
                lm *= 10
                continue
            a2 = np.clip(a + step[:NF], -_AMAX, _AMAX)
            c2 = c + step[NF:]
            Q2, f2 = _solveQ(usamp, Gs, gnorm, a2, c2)
            if f2 < f:
                a, c, Q, f = a2, c2, Q2, f2
                lm = max(lm * 0.3, 1e-8)
                ok = True
                break
            lm *= 10
        if not ok or lm > 1e9:
            break
    return a, c, Q, f


def _fit_basis(coords, w1, b1, w2, b2, NF):
    """Fit g_k(sqrt(u)) ~= sum_f Q[f,k] silu(a_f u + c_f) over the actual
    pairwise-d^2 distribution.  Returns (a fp16-snapped, c, Q, sim_rel)."""
    c64 = coords.astype(np.float64)
    w1 = w1.astype(np.float64).reshape(-1)
    b1 = b1.astype(np.float64)
    w2 = w2.astype(np.float64)
    b2 = b2.astype(np.float64)
    us = []
    for b in range(B):
        cb = c64[b]
        r = (cb * cb).sum(1)
        us.append(np.maximum(r[:, None] + r[None, :] - 2.0 * (cb @ cb.T), 0.0).ravel())
    uall = np.concatenate(us)
    nq = 8192
    usamp = np.quantile(uall, (np.arange(nq) + 0.5) / nq)
    usamp = np.concatenate([usamp, np.zeros(nq // 1024)])  # diagonal at true mass
    Gs = _g_exact(np.sqrt(usamp), w1, b1, w2, b2)
    gnorm = np.linalg.norm(Gs)

    inits = {
        4: [(np.array([0.186, -0.132, 0.519, -1.03]), np.array([2., -2., -1., -14.])),
            (np.array([0.3, -0.3, 1.0, -2.0]), np.array([1., -1., -3., 2.]))],
        8: [(np.array([0.186, -0.132, 0.519, -1.03, 0.024, -0.731, -2.428, -13.477]),
             np.array([2., -2., -1., -14., -2., 1., -14., -14.]))],
    }[NF]
    best = None
    for a0, c0 in inits:
        a, c, Q, f = _varpro(usamp, Gs, gnorm, a0, c0)
        aq = np.float16(a).astype(np.float64)
        Q2, f2 = _solveQ(usamp, Gs, gnorm, aq, c)
        if best is None or f2 < best[0]:
            best = (f2, aq, c, Q2)
    _, a, c, Q = best

    # fp16 end-to-end simulation on batch 0 + all diagonals (norm-weighted)
    u0 = us[0]
    Gt = _g_exact(np.sqrt(u0), w1, b1, w2, b2)
    xq = np.float16(u0).astype(np.float64)
    phi = np.float16(_silu(xq[:, None] * a + c)).astype(np.float64)
    Gf = np.float16(phi @ np.float16(Q).astype(np.float64)).astype(np.float64)
    rel = np.linalg.norm(Gf - Gt) / np.linalg.norm(Gt)
    return a, c, Q, rel


# ------------------------------------------------------------- device data
def _pack_core(coords_b, tlist, avec, cvec, Q, GSZ):
    """Per-core constant tensors for one batch-half (18 tiles)."""
    NF = avec.size
    NGRP = 128 // GSZ
    NQ = 16 * GSZ // 128
    KH = 16 // NQ

    c64 = coords_b.astype(np.float64)
    cf = c64.astype(np.float32)
    ch = _round10(cf)
    cl = (cf.astype(np.float64) - ch).astype(np.float32)
    r = (c64 * c64).sum(1)
    rf = r.astype(np.float32)
    rh = _round10(rf)
    rl = (rf.astype(np.float64) - rh).astype(np.float32)

    cstf = np.zeros((_KD, 2 * NT * 128), np.float32)
    for t, (ci, cj) in enumerate(tlist):
        si = slice(128 * ci, 128 * ci + 128)
        sj = slice(128 * cj, 128 * cj + 128)
        L = np.zeros((_KD, 128), np.float32)
        L[0:3] = -2.0 * ch[si].T
        L[3:6] = -2.0 * ch[si].T
        L[6:9] = -2.0 * cl[si].T
        L[9] = rh[si]
        L[10] = rl[si]
        L[11] = 1.0
        L[12] = 1.0
        R = np.zeros((_KD, 128), np.float32)
        R[0:3] = ch[sj].T
        R[3:6] = cl[sj].T
        R[6:9] = ch[sj].T
        R[9] = 1.0
        R[10] = 1.0
        R[11] = rh[sj]
        R[12] = rl[sj]
        cstf[:, 128 * t:128 * (t + 1)] = L
        cstf[:, NT * 128 + 128 * t:NT * 128 + 128 * (t + 1)] = R

    cst16 = np.zeros((128, (NGRP + NQ) * 128), np.float32)
    p = np.arange(128)
    a_perm = avec[p // GSZ]
    for v in range(NGRP):
        S = np.zeros((128, 128), np.float32)
        S[GSZ * v + (p % GSZ), p] = a_perm
        cst16[:, 128 * v:128 * (v + 1)] = S
    for q in range(NQ):
        Qm = np.zeros((128, 128), np.float32)
        for f in range(NF):
            for kh in range(KH):
                for g in range(GSZ):
                    Qm[GSZ * f + g, GSZ * kh + g] = Q[f, KH * q + kh]
        cst16[:, 128 * (NGRP + q):128 * (NGRP + q + 1)] = Qm
    cst16 = cst16.astype(np.float16)

    cstv = cvec[p // GSZ].astype(np.float32).reshape(128, 1)
    return {"cstf": cstf, "cst16": cst16, "cstv": cstv}


# ---------------------------------------------------------------- program
def _build_program(GSZ):
    import concourse.bacc as bacc
    import concourse.mybir as mybir
    import concourse.tile as tile

    f32 = mybir.dt.float32
    f32r = mybir.dt.float32r
    f16 = mybir.dt.float16
    AF = mybir.ActivationFunctionType

    NGRP = 128 // GSZ
    NQ = 16 * GSZ // 128
    GW = NQ * W                # heads-psum cols per group
    OC = NGRP * GW             # osb cols per stripe
    OUTCOLS = NST * OC

    nc = bacc.Bacc(num_devices=NCORES)
    CSTF = nc.declare_dram_parameter("cstf", [_KD, 2 * NT * 128], f32r, isOutput=False)
    CST16 = nc.declare_dram_parameter("cst16", [128, (NGRP + NQ) * 128], f16, isOutput=False)
    CSTV = nc.declare_dram_parameter("cstv", [128, 1], f32, isOutput=False)
    OUT = nc.declare_dram_parameter("out", [128, OUTCOLS], f16, isOutput=True)

    # out-cast engine split: ACT handles ~10/24 of the psum->sbuf casts
    ncast = NST * NGRP
    nact = max(1, (10 * ncast) // 24)
    pat = []
    acc = 0.0
    for _ in range(ncast):
        acc += nact / ncast
        if acc >= 1.0:
            pat.append('A')
            acc -= 1.0
        else:
            pat.append('D')

    with tile.TileContext(nc) as tc:
        with (
            tc.tile_pool(name="const", bufs=1) as cp,
            tc.tile_pool(name="ut", bufs=2) as utp,
            tc.tile_pool(name="phi", bufs=3) as php,
            tc.tile_pool(name="osb", bufs=2) as osp,
            tc.tile_pool(name="psA", bufs=2, space="PSUM") as psA,
            tc.tile_pool(name="psB", bufs=2, space="PSUM") as psB,
        ):
            ddr = cp.tile([_KD, 2 * NT * 128], f32r, tag="ddr")
            nc.sync.dma_start(ddr[:], CSTF[:])
            c16 = cp.tile([128, (NGRP + NQ) * 128], f16, tag="c16")
            nc.scalar.dma_start(c16[:], CST16[:])
            cv = cp.tile([128, 1], f32, tag="cv")
            nc.scalar.dma_start(cv[:], CSTV[:])

            # silu table load + PE HAM warmup while constants stream in
            warm = cp.tile([128, W], f16, tag="warm")
            nc.gpsimd.memset(warm[:], 0.0)
            warm2 = cp.tile([128, 8], f32, tag="warm2")
            nc.gpsimd.memset(warm2[:], 0.0)
            warm3 = cp.tile([128, 8], f32, tag="warm3")
            nc.scalar.activation(warm3[:], warm2[:], AF.Silu)
            pw = psA.tile([128, W], f32, tag="ps")
            for _ in range(8):
                nc.tensor.matmul(pw[:], warm[:, 0:128], warm[:],
                                 start=True, stop=True)

            ic = 0
            for s in range(NST):
                pd = psA.tile([128, W], f32, tag="ps")
                for u3 in range(NSB):
                    t = NSB * s + u3
                    nc.tensor.matmul(
                        pd[:, 128 * u3:128 * (u3 + 1)],
                        ddr[:, 128 * t:128 * (t + 1)],
                        ddr[:, NT * 128 + 128 * t:NT * 128 + 128 * (t + 1)],
                        start=True, stop=True)
                ut = utp.tile([128, W], f16, tag="ut")
                nc.vector.tensor_copy(ut[:], pd[:])
                osb = osp.tile([128, OC], f16, tag="osb")
                for v in range(NGRP):
                    pph = psA.tile([128, W], f32, tag="ps")
                    nc.tensor.matmul(pph[:], c16[:, 128 * v:128 * (v + 1)],
                                     ut[:], start=True, stop=True)
                    phi = php.tile([128, W], f16, tag="phi")
                    nc.scalar.activation(phi[:], pph[:], AF.Silu,
                                         bias=cv[:, 0:1], scale=1.0)
                    po = psB.tile([128, GW], f32, tag="po")
                    for q in range(NQ):
                        # split at PSUM bank boundaries: one matmul output
                        # must stay within a single 512-col bank
                        x0 = W * q
                        while x0 < W * (q + 1):
                            x1 = min(W * (q + 1), (x0 // 512 + 1) * 512)
                            nc.tensor.matmul(
                                po[:, x0:x1],
                                c16[:, 128 * (NGRP + q):128 * (NGRP + q + 1)],
                                phi[:, x0 - W * q:x1 - W * q],
                                start=True, stop=True)
                            x0 = x1
                    dst = osb[:, GW * v:GW * (v + 1)]
                    if pat[ic] == 'A':
                        nc.scalar.copy(dst, po[:])
                    else:
                        nc.vector.tensor_copy(dst, po[:])
                    ic += 1
                eng = (nc.sync, nc.gpsimd)[s % 2]
                eng.dma_start(OUT[:, OC * s:OC * (s + 1)], osb[:])
    nc.compile()
    return nc


# -------------------------------------------------------------------- run
def _run(coords, w1, b1, w2, b2, trace=False):
    from concourse.bass_utils import run_bass_kernel_spmd

    coords = np.asarray(coords)
    avec, cvec, Q, rel = _fit_basis(coords, w1, b1, w2, b2, 4)
    GSZ = 32
    if rel > 1.2e-2:   # fallback: richer basis, 16-row groups
        avec, cvec, Q, rel = _fit_basis(coords, w1, b1, w2, b2, 8)
        GSZ = 16

    if GSZ not in _prog_cache:
        _prog_cache[GSZ] = _build_program(GSZ)
    nc = _prog_cache[GSZ]

    in_maps = []
    for core in range(NCORES):
        b, half = divmod(core, 2)
        tlist = TILES[18 * half:18 * (half + 1)]
        in_maps.append(_pack_core(coords[b], tlist, avec, cvec, Q, GSZ))

    res = run_bass_kernel_spmd(nc, in_maps, list(range(NCORES)), trace=trace)

    NGRP = 128 // GSZ
    NQ = 16 * GSZ // 128
    KH = 16 // NQ
    out = np.empty((B, HEADS, N, N), dtype=np.float32)
    for core in range(NCORES):
        b, half = divmod(core, 2)
        tlist = TILES[18 * half:18 * (half + 1)]
        raw = res.results[core]["out"]
        # [m, col] -> [kh, g, s, v, q, u, jj]
        A = raw.reshape(KH, GSZ, NST, NGRP, NQ, NSB, 128)
        for t, (ci, cj) in enumerate(tlist):
            s, u3 = divmod(t, NSB)
            blk = A[:, :, s, :, :, u3, :]            # [kh, g, v, q, jj]
            tl = blk.transpose(3, 0, 2, 1, 4).reshape(HEADS, 128, 128)
            i0, j0 = 128 * ci, 128 * cj
            out[b, :, i0:i0 + 128, j0:j0 + 128] = tl
            if ci != cj:
                out[b, :, j0:j0 + 128, i0:i0 + 128] = tl.transpose(0, 2, 1)
    return out, res


def kernel(coords, w1, b1, w2, b2):
    out, _ = _run(coords, w1, b1, w2, b2, trace=False)
    return out


# revision 6
# speedup vs baseline: 1.7681x; 1.1313x over previous
"""Trainium2 kernel for nn_DistanceRelativeBias.

Computes out[b,k,i,j] = g_k(||c_i - c_j||) where g_k(d) = b2[k] +
sum_h w2[h,k]*silu(w1[h]*d + b1[h]).

Key ideas vs the previous version (115 us):
  1. SYMMETRY: out[b,k,i,j] == out[b,k,j,i], so only the upper-triangle
     128x128 (i,j) tiles are computed on device (36 of 64 per batch);
     the host mirrors the off-diagonal tiles.  144 tiles / 8 cores = 18
     tiles per core, perfectly balanced.  Tile coordinates are baked
     into per-core constant data (the lhs/rhs columns of the d2 matmul),
     so all cores run one SPMD program.
  2. d^2 DOMAIN: the 16 head-functions are refit (runtime VarPro
     Gauss-Newton, residual ~5e-3) onto an NF-term silu basis in
     u = d^2:  g_k ~= sum_f Q[f,k]*silu(a_f*u + c_f).  No sqrt pass,
     no ACT table switch, no clamp (silu of a slightly-negative u is
     benign, unlike sqrt).
  3. NF=4 basis with 32-row groups: silu cost on ACT scales with
     NF (phi elements), so fewer, wider groups cut ACT work ~2.4x.
     (Falls back to NF=8 / 16-row groups if the runtime fit is poor.)

Per core, per 384-column stripe (3 of its 18 tiles):
  PE  : u[i,j] = -2 c_i.c_j + r_i + r_j   (K=13 hi/lo-split f32r matmul)
  DVE : psum -> fp16 u-tile
  per 32-row group v: PE broadcast-matmul (selector scatters rows across
        partitions p=32f+g with weight a_f) -> ACT silu(.+c_f) -> fp16
        phi -> PE heads matmuls (4x 4-head blocks, m=32k+g) ->
        DVE/ACT cast psum -> fp16 osb
  DMA : osb [128, 6144] fp16 -> DRAM (1.5 MB contiguous, 12KB runs)
Host unscrambles (pure data movement) + mirrors + upcasts to fp32.
"""
import numpy as np

B, N, D = 4, 1024, 3
HID, HEADS = 64, 16
NCORES = 8
NT = 18                 # 128x128 tiles per core
NSB = 3                 # tile-subblocks per stripe
NST = NT // NSB         # stripes per core (6)
W = NSB * 128           # stripe width (384)
_KD = 13                # d2 matmul contraction (hi/lo split for f32r)
_AMAX = 24.0            # silu slope bound (fp16-robustness of the basis)

# upper-triangle tile list per batch: 36 tiles; cores 2b, 2b+1 take halves
TILES = [(ci, cj) for ci in range(8) for cj in range(ci, 8)]

_prog_cache = {}


def _round10(x):
    """Round mantissa to 10 bits (conservative f32r grid)."""
    i = np.asarray(x, np.float32).view(np.int32).astype(np.int64)
    r = (i + 0x1000 + ((i >> 13) & 1)) >> 13 << 13
    return (r & 0xFFFFFFFF).astype(np.uint32).view(np.float32)


def _silu(x):
    x = np.clip(x, -60.0, 60.0)
    return x / (1.0 + np.exp(-x))


# ---------------------------------------------------------------- basis fit
def _g_exact(d, w1, b1, w2, b2):
    return _silu(d[..., None] * w1 + b1) @ w2 + b2


def _solveQ(usamp, Gs, gnorm, a, c, lam=1e-10):
    phi = _silu(usamp[:, None] * a + c)
    A = phi.T @ phi + lam * np.eye(a.size)
    try:
        Q = np.linalg.solve(A, phi.T @ Gs)
    except np.linalg.LinAlgError:
        Q = np.linalg.lstsq(phi, Gs, rcond=None)[0]
    return Q, np.linalg.norm(phi @ Q - Gs) / gnorm


def _varpro(usamp, Gs, gnorm, a0, c0, iters=80):
    """Variable-projection Gauss-Newton over (a, c); Q solved exactly."""
    a, c = a0.astype(np.float64).copy(), c0.astype(np.float64).copy()
    NF = a.size
    Q, f = _solveQ(usamp, Gs, gnorm, a, c)
    lm = 1e-3
    for _ in range(iters):
        p = np.concatenate([a, c])
        r0 = (_silu(usamp[:, None] * a + c) @ Q - Gs).ravel()
        J = np.empty((r0.size, 2 * NF))
        for i in range(2 * NF):
            dp = np.zeros(2 * NF)
            dp[i] = max(1e-5, 1e-6 * abs(p[i]))
            a2 = np.clip((p + dp)[:NF], -_AMAX, _AMAX)
            c2 = (p + dp)[NF:]
            Q2, _ = _solveQ(usamp, Gs, gnorm, a2, c2)
            J[:, i] = ((_silu(usamp[:, None] * a2 + c2) @ Q2 - Gs).ravel() - r0) / dp[i]
        JtJ = J.T @ J
        Jtr = J.T @ r0
        ok = False
        for _ in range(10):
            try:
                step = np.linalg.solve(
                    JtJ + lm * np.diag(np.maximum(np.diag(JtJ), 1e-12)), -Jtr)
            except np.linalg.LinAlgError:
                lm *= 10
                continue
            a2 = np.clip(a + step[:NF], -_AMAX, _AMAX)
            c2 = c + step[NF:]
            Q2, f2 = _solveQ(usamp, Gs, gnorm, a2, c2)
            if f2 < f:
                a, c, Q, f = a2, c2, Q2, f2
                lm = max(lm * 0.3, 1e-8)
                ok = True
                break
            lm *= 10
        if not ok or lm > 1e9:
            break
    return a, c, Q, f


def _fit_basis(coords, w1, b1, w2, b2, NF):
    """Fit g_k(sqrt(u)) ~= sum_f Q[f,k] silu(a_f u + c_f) over the actual
    pairwise-d^2 distribution.  Returns (a fp16-snapped, c, Q, sim_rel)."""
    c64 = coords.astype(np.float64)
    w1 = w1.astype(np.float64).reshape(-1)
    b1 = b1.astype(np.float64)
    w2 = w2.astype(np.float64)
    b2 = b2.astype(np.float64)
    us = []
    for b in range(B):
        cb = c64[b]
        r = (cb * cb).sum(1)
        us.append(np.maximum(r[:, None] + r[None, :] - 2.0 * (cb @ cb.T), 0.0).ravel())
    uall = np.concatenate(us)
    nq = 8192
    usamp = np.quantile(uall, (np.arange(nq) + 0.5) / nq)
    usamp = np.concatenate([usamp, np.zeros(nq // 1024)])  # diagonal at true mass
    Gs = _g_exact(np.sqrt(usamp), w1, b1, w2, b2)
    gnorm = np.linalg.norm(Gs)

    inits = {
        4: [(np.array([0.186, -0.132, 0.519, -1.03]), np.array([2., -2., -1., -14.])),
            (np.array([0.3, -0.3, 1.0, -2.0]), np.array([1., -1., -3., 2.]))],
        8: [(np.array([0.186, -0.132, 0.519, -1.03, 0.024, -0.731, -2.428, -13.477]),
             np.array([2., -2., -1., -14., -2., 1., -14., -14.]))],
    }[NF]
    best = None
    for a0, c0 in inits:
        a, c, Q, f = _varpro(usamp, Gs, gnorm, a0, c0)
        aq = np.float16(a).astype(np.float64)
        Q2, f2 = _solveQ(usamp, Gs, gnorm, aq, c)
        if best is None or f2 < best[0]:
            best = (f2, aq, c, Q2)
    _, a, c, Q = best

    # fp16 end-to-end simulation on batch 0 + all diagonals (norm-weighted)
    u0 = us[0]
    Gt = _g_exact(np.sqrt(u0), w1, b1, w2, b2)
    xq = np.float16(u0).astype(np.float64)
    phi = np.float16(_silu(xq[:, None] * a + c)).astype(np.float64)
    Gf = np.float16(phi @ np.float16(Q).astype(np.float64)).astype(np.float64)
    rel = np.linalg.norm(Gf - Gt) / np.linalg.norm(Gt)
    return a, c, Q, rel


# ------------------------------------------------------------- device data
def _pack_core(coords_b, tlist, avec, cvec, Q, GSZ):
    """Per-core constant tensors for one batch-half (18 tiles)."""
    NF = avec.size
    NGRP = 128 // GSZ
    NQ = 16 * GSZ // 128
    KH = 16 // NQ

    c64 = coords_b.astype(np.float64)
    cf = c64.astype(np.float32)
    ch = _round10(cf)
    cl = (cf.astype(np.float64) - ch).astype(np.float32)
    r = (c64 * c64).sum(1)
    rf = r.astype(np.float32)
    rh = _round10(rf)
    rl = (rf.astype(np.float64) - rh).astype(np.float32)

    cstf = np.zeros((_KD, 2 * NT * 128), np.float32)
    for t, (ci, cj) in enumerate(tlist):
        si = slice(128 * ci, 128 * ci + 128)
        sj = slice(128 * cj, 128 * cj + 128)
        L = np.zeros((_KD, 128), np.float32)
        L[0:3] = -2.0 * ch[si].T
        L[3:6] = -2.0 * ch[si].T
        L[6:9] = -2.0 * cl[si].T
        L[9] = rh[si]
        L[10] = rl[si]
        L[11] = 1.0
        L[12] = 1.0
        R = np.zeros((_KD, 128), np.float32)
        R[0:3] = ch[sj].T
        R[3:6] = cl[sj].T
        R[6:9] = ch[sj].T
        R[9] = 1.0
        R[10] = 1.0
        R[11] = rh[sj]
        R[12] = rl[sj]
        cstf[:, 128 * t:128 * (t + 1)] = L
        cstf[:, NT * 128 + 128 * t:NT * 128 + 128 * (t + 1)] = R

    cst16 = np.zeros((128, (NGRP + NQ) * 128), np.float32)
    p = np.arange(128)
    a_perm = avec[p // GSZ]
    for v in range(NGRP):
        S = np.zeros((128, 128), np.float32)
        S[GSZ * v + (p % GSZ), p] = a_perm
        cst16[:, 128 * v:128 * (v + 1)] = S
    for q in range(NQ):
        Qm = np.zeros((128, 128), np.float32)
        for f in range(NF):
            for kh in range(KH):
                for g in range(GSZ):
                    Qm[GSZ * f + g, GSZ * kh + g] = Q[f, KH * q + kh]
        cst16[:, 128 * (NGRP + q):128 * (NGRP + q + 1)] = Qm
    cst16 = cst16.astype(np.float16)

    cstv = cvec[p // GSZ].astype(np.float32).reshape(128, 1)
    return {"cstf": cstf, "cst16": cst16, "cstv": cstv}


# ---------------------------------------------------------------- program
def _build_program(GSZ):
    import concourse.bacc as bacc
    import concourse.mybir as mybir
    import concourse.tile as tile

    f32 = mybir.dt.float32
    f32r = mybir.dt.float32r
    f16 = mybir.dt.float16
    AF = mybir.ActivationFunctionType

    NGRP = 128 // GSZ
    NQ = 16 * GSZ // 128
    GW = NQ * W                # heads-psum cols per group
    OC = NGRP * GW             # osb cols per stripe
    OUTCOLS = NST * OC

    nc = bacc.Bacc(num_devices=NCORES)
    CSTF = nc.declare_dram_parameter("cstf", [_KD, 2 * NT * 128], f32r, isOutput=False)
    CST16 = nc.declare_dram_parameter("cst16", [128, (NGRP + NQ) * 128], f16, isOutput=False)
    CSTV = nc.declare_dram_parameter("cstv", [128, 1], f32, isOutput=False)
    OUT = nc.declare_dram_parameter("out", [128, OUTCOLS], f16, isOutput=True)

    # out-cast engine split: ACT handles ~10/24 of the psum->sbuf casts
    ncast = NST * NGRP
    nact = max(1, (10 * ncast) // 24)
    pat = []
    acc = 0.0
    for _ in range(ncast):
        acc += nact / ncast
        if acc >= 1.0:
            pat.append('A')
            acc -= 1.0
        else:
            pat.append('D')

    with tile.TileContext(nc) as tc:
        with (
            tc.tile_pool(name="const", bufs=1) as cp,
            tc.tile_pool(name="ut", bufs=2) as utp,
            tc.tile_pool(name="phi", bufs=3) as php,
            tc.tile_pool(name="osb", bufs=2) as osp,
            tc.tile_pool(name="psA", bufs=2, space="PSUM") as psA,
            tc.tile_pool(name="psB", bufs=2, space="PSUM") as psB,
        ):
            ddr = cp.tile([_KD, 2 * NT * 128], f32r, tag="ddr")
            nc.sync.dma_start(ddr[:], CSTF[:])
            c16 = cp.tile([128, (NGRP + NQ) * 128], f16, tag="c16")
            nc.scalar.dma_start(c16[:], CST16[:])
            cv = cp.tile([128, 1], f32, tag="cv")
            nc.scalar.dma_start(cv[:], CSTV[:])

            # silu table load + PE HAM warmup while constants stream in
            warm = cp.tile([128, W], f16, tag="warm")
            nc.vector.memset(warm[:], 0.0)
            warm2 = cp.tile([128, 8], f32, tag="warm2")
            nc.vector.memset(warm2[:], 0.0)
            warm3 = cp.tile([128, 8], f32, tag="warm3")
            nc.scalar.activation(warm3[:], warm2[:], AF.Silu)
            pw = psA.tile([128, W], f32, tag="ps")
            for _ in range(10):
                nc.tensor.matmul(pw[:], warm[:, 0:128], warm[:],
                                 start=True, stop=True)

            # software-pipelined main loop: broadcast of group g+1 is issued
            # to the PE before heads of group g, so the PE never waits on the
            # ACT silu; osb halves DMA out as soon as their casts land
            NG = NST * NGRP
            pdt = [None] * NST
            utt = [None] * NST
            osbt = [None] * NST
            pht = [None] * NG
            pot = [None] * NG
            ndma = 0
            for g in range(NG + 1):
                if g < NG:
                    s, v = divmod(g, NGRP)
                    if v == 0:
                        pd = psA.tile([128, W], f32, tag="ps")
                        for u3 in range(NSB):
                            t = NSB * s + u3
                            nc.tensor.matmul(
                                pd[:, 128 * u3:128 * (u3 + 1)],
                                ddr[:, 128 * t:128 * (t + 1)],
                                ddr[:, NT * 128 + 128 * t:NT * 128 + 128 * (t + 1)],
                                start=True, stop=True)
                        ut = utp.tile([128, W], f16, tag="ut")
                        nc.vector.tensor_copy(ut[:], pd[:])
                        utt[s] = ut
                        osb = osp.tile([128, OC], f16, tag="osb")
                        osbt[s] = osb
                    pph = psA.tile([128, W], f32, tag="ps")
                    nc.tensor.matmul(pph[:], c16[:, 128 * v:128 * (v + 1)],
                                     utt[s][:], start=True, stop=True)
                    phi = php.tile([128, W], f16, tag="phi")
                    nc.scalar.activation(phi[:], pph[:], AF.Silu,
                                         bias=cv[:, 0:1], scale=1.0)
                    pht[g] = phi
                if g >= 1:
                    h = g - 1
                    sh, vh = divmod(h, NGRP)
                    po = psB.tile([128, GW], f32, tag="po")
                    for q in range(NQ):
                        # split at PSUM bank boundaries: one matmul output
                        # must stay within a single 512-col bank
                        x0 = W * q
                        while x0 < W * (q + 1):
                            x1 = min(W * (q + 1), (x0 // 512 + 1) * 512)
                            nc.tensor.matmul(
                                po[:, x0:x1],
                                c16[:, 128 * (NGRP + q):128 * (NGRP + q + 1)],
                                pht[h][:, x0 - W * q:x1 - W * q],
                                start=True, stop=True)
                            x0 = x1
                    dst = osbt[sh][:, GW * vh:GW * (vh + 1)]
                    if pat[h] == 'A':
                        nc.scalar.copy(dst, po[:])
                    else:
                        nc.vector.tensor_copy(dst, po[:])
                    if vh == NGRP // 2 - 1 or vh == NGRP - 1:
                        half = 0 if vh < NGRP // 2 else 1
                        hw = OC // 2
                        eng = (nc.sync, nc.gpsimd)[ndma % 2]
                        ndma += 1
                        eng.dma_start(
                            OUT[:, OC * sh + hw * half:OC * sh + hw * (half + 1)],
                            osbt[sh][:, hw * half:hw * (half + 1)])
    nc.compile()
    return nc


# -------------------------------------------------------------------- run
def _run(coords, w1, b1, w2, b2, trace=False):
    from concourse.bass_utils import run_bass_kernel_spmd

    coords = np.asarray(coords)
    avec, cvec, Q, rel = _fit_basis(coords, w1, b1, w2, b2, 4)
    GSZ = 32
    if rel > 1.2e-2:   # fallback: richer basis, 16-row groups
        avec, cvec, Q, rel = _fit_basis(coords, w1, b1, w2, b2, 8)
        GSZ = 16

    if GSZ not in _prog_cache:
        _prog_cache[GSZ] = _build_program(GSZ)
    nc = _prog_cache[GSZ]

    in_maps = []
    for core in range(NCORES):
        b, half = divmod(core, 2)
        tlist = TILES[18 * half:18 * (half + 1)]
        in_maps.append(_pack_core(coords[b], tlist, avec, cvec, Q, GSZ))

    res = run_bass_kernel_spmd(nc, in_maps, list(range(NCORES)), trace=trace)

    NGRP = 128 // GSZ
    NQ = 16 * GSZ // 128
    KH = 16 // NQ
    out = np.empty((B, HEADS, N, N), dtype=np.float32)
    for core in range(NCORES):
        b, half = divmod(core, 2)
        tlist = TILES[18 * half:18 * (half + 1)]
        raw = res.results[core]["out"]
        # [m, col] -> [kh, g, s, v, q, u, jj]
        A = raw.reshape(KH, GSZ, NST, NGRP, NQ, NSB, 128)
        for t, (ci, cj) in enumerate(tlist):
            s, u3 = divmod(t, NSB)
            blk = A[:, :, s, :, :, u3, :]            # [kh, g, v, q, jj]
            tl = blk.transpose(3, 0, 2, 1, 4).reshape(HEADS, 128, 128)
            i0, j0 = 128 * ci, 128 * cj
            out[b, :, i0:i0 + 128, j0:j0 + 128] = tl
            if ci != cj:
                out[b, :, j0:j0 + 128, i0:i0 + 128] = tl.transpose(0, 2, 1)
    return out, res


def kernel(coords, w1, b1, w2, b2):
    out, _ = _run(coords, w1, b1, w2, b2, trace=False)
    return out
